# revision 5
# baseline (speedup 1.0000x reference)
"""Trainium2 Bass kernel for a FlowNet-style CorrelationLayer.

out[0, j*7+i, h, w] = sum_c x[0,c,h,w] * y[0,c,h+j-3,w+i-3]   (zero-padded y)

Shapes: x, y = [1, 128, 384, 512] fp32  ->  out = [1, 49, 384, 512] fp32.

Strategy
--------
* Shard H (rows) across the 8 NeuronCores: core k computes output rows
  [48k, 48k+48).  The y halo (3 rows each side) is sliced on the host from
  the full input, so no inter-core communication is needed.
* Per core, the C=128 contraction runs on the TensorEngine as "all-pairs"
  patch matmuls: lhsT = an 8x16 pixel patch of x (M=128 columns, K=C=128),
  rhs = the matching 14x22 halo patch of y (N=308 columns).  Entry
  (m=(a,b), n=(al,be)) of the PSUM block is the correlation of x pixel
  (a,b) with y pixel (al-3, be-3) relative to the patch origin, so the 49
  shift planes live on 49 diagonals of each block.
* Diagonal extraction is not expressible with uniform per-partition access
  patterns on any engine, so each PSUM block is cast to fp16 and dumped
  whole to DRAM; the final banded gather is a cheap numpy fancy-index on
  the host.  Inputs are also shipped as fp16 (quantization error ~1e-3
  relative, well within tolerance), which keeps total HBM traffic per core
  at ~28 MB, close to the memory roofline.
"""

import numpy as np

import concourse.bass as bass  # noqa: F401  (AP types pulled in transitively)
import concourse.tile as tile
from concourse import bacc, mybir
from concourse.bass_utils import run_bass_kernel_spmd

B, C, H, W = 1, 128, 384, 512
NCORES = 8
HB = H // NCORES          # 48 output rows per core
PA, PB = 8, 16            # x patch: 8 rows x 16 cols = 128 = M
HA, HW_ = PA + 6, PB + 6  # y halo patch: 14 x 22
PR, PW = HB // PA, W // PB  # 6 patch-rows x 32 patch-cols = 192 patches
NF = HA * HW_             # 308 = N (matmul free size, <= 512)

F16 = mybir.dt.float16

_PROGRAM = None


def _build_program():
    nc = bacc.Bacc("TRN2", target_bir_lowering=False, debug=False)

    # x is pre-tiled on the host to [C, patch, m] so each patch's 128 weight
    # columns are contiguous (walrus requires a single free dim on the
    # stationary matmul operand).
    xb = nc.declare_dram_parameter("xb", [C, PR * PW, PA * PB], F16, isOutput=False)
    yb = nc.declare_dram_parameter("yb", [C, HB + 6, W + 6], F16, isOutput=False)
    corr = nc.declare_dram_parameter("corr", [PR, PW, 128, NF], F16, isOutput=True)

    with tile.TileContext(nc) as tc:
        with (
            tc.tile_pool(name="xpool", bufs=1) as xpool,
            tc.tile_pool(name="ypool", bufs=1) as ypool,
            tc.tile_pool(name="psum", bufs=8, space="PSUM") as psum_pool,
            tc.tile_pool(name="stage", bufs=8) as stage_pool,
        ):
            X = xpool.tile([C, PR * PW, PA * PB], F16)
            Y = ypool.tile([C, HB + 6, W + 6], F16)

            # Chunked loads so several DMA queues run in parallel.
            for pr in range(PR):
                nc.sync.dma_start(
                    X[:, pr * PW : (pr + 1) * PW, :], xb[:, pr * PW : (pr + 1) * PW, :]
                )
            for r in range(0, HB + 6, 9):
                nc.sync.dma_start(Y[:, r : r + 9, :], yb[:, r : r + 9, :])

            for pr in range(PR):
                for wp in range(PW):
                    lhsT = X[:, pr * PW + wp, :]
                    rhs = Y[:, pr * PA : pr * PA + HA, wp * PB : wp * PB + HW_]
                    ps = psum_pool.tile([128, NF], mybir.dt.float32)
                    nc.tensor.matmul(ps[:], lhsT, rhs, start=True, stop=True)
                    st = stage_pool.tile([128, NF], F16)
                    # Alternate evacuation between DVE and ACT so neither
                    # becomes the bottleneck.
                    if (pr * PW + wp) % 2 == 0:
                        nc.vector.tensor_copy(st[:], ps[:])
                    else:
                        nc.scalar.copy(st[:], ps[:])
                    nc.sync.dma_start(corr[pr, wp], st[:])

    nc.compile()
    return nc


def _program():
    global _PROGRAM
    if _PROGRAM is None:
        _PROGRAM = _build_program()
    return _PROGRAM


def _make_in_maps(x: np.ndarray, y: np.ndarray):
    x0 = np.asarray(x[0]).astype(np.float16)
    # [C, H, W] -> [C, PRtot, PA, PW, PB] -> [C, PRtot, PW, PA, PB]
    xt = x0.reshape(C, H // PA, PA, PW, PB).transpose(0, 1, 3, 2, 4)
    xt = np.ascontiguousarray(xt.reshape(C, H // PA * PW, PA * PB))
    yp = np.zeros((C, H + 6, W + 6), np.float16)
    yp[:, 3 : 3 + H, 3 : 3 + W] = y[0]
    in_maps = []
    for k in range(NCORES):
        in_maps.append(
            {
                "xb": np.ascontiguousarray(xt[:, k * PR * PW : (k + 1) * PR * PW, :]),
                "yb": np.ascontiguousarray(yp[:, k * HB : k * HB + HB + 6, :]),
            }
        )
    return in_maps


_GATHER_IDX = None


def _gather_indices():
    global _GATHER_IDX
    if _GATHER_IDX is None:
        a = np.arange(PA)[:, None]
        b = np.arange(PB)[None, :]
        m_idx = a * PB + b  # [8, 16]
        j = np.arange(7)[:, None, None, None]
        i = np.arange(7)[None, :, None, None]
        n_idx = (a[None, None] + j) * HW_ + (b[None, None] + i)  # [7, 7, 8, 16]
        _GATHER_IDX = (m_idx[None, None], n_idx)
    return _GATHER_IDX


def _gather_core(corr_k: np.ndarray) -> np.ndarray:
    """[PR, PW, 128, NF] -> [49, HB, W] band of the output."""
    m_idx, n_idx = _gather_indices()
    g = corr_k[:, :, m_idx, n_idx]  # [PR, PW, 7, 7, 8, 16]
    g = g.transpose(2, 3, 0, 4, 1, 5).reshape(49, HB, W)
    return g


def _run(in_maps, trace=False, **kw):
    return run_bass_kernel_spmd(
        _program(), in_maps, core_ids=list(range(NCORES)), trace=trace, **kw
    )


def kernel(x: np.ndarray, y: np.ndarray) -> np.ndarray:
    x = np.asarray(x)
    y = np.asarray(y)
    res = _run(_make_in_maps(x, y)).results
    out = np.empty((1, 49, H, W), np.float32)
    for k in range(NCORES):
        out[0, :, k * HB : (k + 1) * HB, :] = _gather_core(
            np.asarray(res[k]["corr"])
        ).astype(np.float32)
    return out


# revision 10
# speedup vs baseline: 1.8260x; 1.8260x over previous
"""Trainium2 Bass kernel for a FlowNet-style CorrelationLayer.

out[0, j*7+i, h, w] = sum_c x[0,c,h,w] * y[0,c,h+j-3,w+i-3]   (zero-padded y)

Shapes: x, y = [1, 128, 384, 512] fp32  ->  out = [1, 49, 384, 512] fp32.

Strategy
--------
* Shard H (rows) across the 8 NeuronCores: core k computes output rows
  [48k, 48k+48).  The y halo (3 rows each side) is sliced on the host from
  the full input, so no inter-core communication is needed.
* Per core, the C=128 contraction runs on the TensorEngine as "all-pairs"
  patch matmuls: lhsT = an 8x16 pixel patch of x (M=128 columns, K=C=128),
  rhs = the matching 14x22 halo patch of y (N=308 columns).  Entry
  (m=(a,b), n=(al,be)) of the PSUM block is the correlation of x pixel
  (a,b) with y pixel (al-3, be-3) relative to the patch origin, so the 49
  shift planes live on 49 diagonals of each block.
* Diagonal extraction is not expressible with uniform per-partition access
  patterns on any engine, so each PSUM block is cast to fp16 and dumped
  whole to DRAM; the final banded gather is a cheap numpy fancy-index on
  the host.  Inputs are also shipped as fp16 (quantization error ~1e-3
  relative, well within tolerance), which keeps total HBM traffic per core
  at ~28 MB, close to the memory roofline.
"""

import numpy as np

import concourse.bass as bass  # noqa: F401  (AP types pulled in transitively)
import concourse.tile as tile
from concourse import bacc, mybir
from concourse.bass_utils import run_bass_kernel_spmd

B, C, H, W = 1, 128, 384, 512
NCORES = 8
HB = H // NCORES          # 48 output rows per core
PA, PB = 8, 16            # x patch: 8 rows x 16 cols = 128 = M
HA, HW_ = PA + 6, PB + 6  # y halo patch: 14 x 22
PR, PW = HB // PA, W // PB  # 6 patch-rows x 32 patch-cols = 192 patches
NF = HA * HW_             # 308 = N (matmul free size, <= 512)

F16 = mybir.dt.float16

_PROGRAM = None


def _build_program():
    nc = bacc.Bacc("TRN2", target_bir_lowering=False, debug=False)

    # x is pre-tiled on the host to [C, patch, m] so each patch's 128 weight
    # columns are contiguous (walrus requires a single free dim on the
    # stationary matmul operand).
    xb = nc.declare_dram_parameter("xb", [C, PR * PW, PA * PB], F16, isOutput=False)
    yb = nc.declare_dram_parameter("yb", [C, HB + 6, W + 6], F16, isOutput=False)
    corr = nc.declare_dram_parameter("corr", [PR, 128, PW, NF], F16, isOutput=True)

    with tile.TileContext(nc) as tc:
        with (
            tc.tile_pool(name="xpool", bufs=1) as xpool,
            tc.tile_pool(name="ypool", bufs=1) as ypool,
            tc.tile_pool(name="psum", bufs=4, space="PSUM") as psum_pool,
            tc.tile_pool(name="stage", bufs=2) as stage_pool,
        ):
            X = xpool.tile([C, PR * PW, PA * PB], F16)
            Y = ypool.tile([C, HB + 6, W + 6], F16)

            # Chunked loads so several DMA queues run in parallel.
            for pr in range(PR):
                nc.sync.dma_start(
                    X[:, pr * PW : (pr + 1) * PW, :], xb[:, pr * PW : (pr + 1) * PW, :]
                )
            for r in range(0, HB + 6, 9):
                nc.sync.dma_start(Y[:, r : r + 9, :], yb[:, r : r + 9, :])

            for pr in range(PR):
                # One staging buffer and one output DMA per patch-row keeps
                # the Sync sequencer's per-DMA dispatch (~0.6us) off the
                # critical path.
                st = stage_pool.tile([128, PW, NF], F16)
                for wq in range(PW // 2):
                    # Two patches share one 2-bank PSUM tile; their
                    # evacuation is a single strided copy.
                    ps = psum_pool.tile([128, 2, 512], mybir.dt.float32)
                    for k in range(2):
                        wp = wq * 2 + k
                        lhsT = X[:, pr * PW + wp, :]
                        rhs = Y[:, pr * PA : pr * PA + HA, wp * PB : wp * PB + HW_]
                        nc.tensor.matmul(
                            ps[:, k, :NF], lhsT, rhs, start=True, stop=True
                        )
                    dst = st[:, wq * 2 : wq * 2 + 2, :]
                    # Alternate evacuation between DVE and ACT so neither
                    # becomes the bottleneck.
                    if wq % 2 == 0:
                        nc.vector.tensor_copy(dst, ps[:, :, :NF])
                    else:
                        nc.scalar.copy(dst, ps[:, :, :NF])
                nc.sync.dma_start(corr[pr], st[:])

    nc.compile()
    return nc


def _program():
    global _PROGRAM
    if _PROGRAM is None:
        _PROGRAM = _build_program()
    return _PROGRAM


def _make_in_maps(x: np.ndarray, y: np.ndarray):
    x0 = np.asarray(x[0]).astype(np.float16)
    # [C, H, W] -> [C, PRtot, PA, PW, PB] -> [C, PRtot, PW, PA, PB]
    xt = x0.reshape(C, H // PA, PA, PW, PB).transpose(0, 1, 3, 2, 4)
    xt = np.ascontiguousarray(xt.reshape(C, H // PA * PW, PA * PB))
    yp = np.zeros((C, H + 6, W + 6), np.float16)
    yp[:, 3 : 3 + H, 3 : 3 + W] = y[0]
    in_maps = []
    for k in range(NCORES):
        in_maps.append(
            {
                "xb": np.ascontiguousarray(xt[:, k * PR * PW : (k + 1) * PR * PW, :]),
                "yb": np.ascontiguousarray(yp[:, k * HB : k * HB + HB + 6, :]),
            }
        )
    return in_maps


_GATHER_IDX = None


def _gather_indices():
    global _GATHER_IDX
    if _GATHER_IDX is None:
        a = np.arange(PA)[:, None]
        b = np.arange(PB)[None, :]
        m_idx = a * PB + b  # [8, 16]
        j = np.arange(7)[:, None, None, None]
        i = np.arange(7)[None, :, None, None]
        n_idx = (a[None, None] + j) * HW_ + (b[None, None] + i)  # [7, 7, 8, 16]
        _GATHER_IDX = (m_idx[None, None], n_idx)
    return _GATHER_IDX


def _gather_core(corr_k: np.ndarray) -> np.ndarray:
    """[PR, 128, PW, NF] -> [49, HB, W] band of the output."""
    m_idx, n_idx = _gather_indices()
    # advanced indices (m, n) separated by a slice -> their axes come first
    g = corr_k[:, m_idx[0, 0], :, n_idx]  # [7, 7, 8, 16, PR, PW]
    g = g.transpose(0, 1, 4, 2, 5, 3).reshape(49, HB, W)
    return g


def _run(in_maps, trace=False, **kw):
    return run_bass_kernel_spmd(
        _program(), in_maps, core_ids=list(range(NCORES)), trace=trace, **kw
    )


def kernel(x: np.ndarray, y: np.ndarray) -> np.ndarray:
    x = np.asarray(x)
    y = np.asarray(y)
    res = _run(_make_in_maps(x, y)).results
    out = np.empty((1, 49, H, W), np.float32)
    for k in range(NCORES):
        out[0, :, k * HB : (k + 1) * HB, :] = _gather_core(
            np.asarray(res[k]["corr"])
        ).astype(np.float32)
    return out


# revision 13
# speedup vs baseline: 2.2250x; 1.2185x over previous
"""Trainium2 Bass kernel for a FlowNet-style CorrelationLayer.

out[0, j*7+i, h, w] = sum_c x[0,c,h,w] * y[0,c,h+j-3,w+i-3]   (zero-padded y)

Shapes: x, y = [1, 128, 384, 512] fp32  ->  out = [1, 49, 384, 512] fp32.

Strategy
--------
* Shard H (rows) across the 8 NeuronCores: core k computes output rows
  [48k, 48k+48).  The y halo (3 rows each side) is sliced on the host from
  the full input, so no inter-core communication is needed.
* Per core, the C=128 contraction runs on the TensorEngine as "all-pairs"
  patch matmuls: lhsT = an 8x16 pixel patch of x (M=128 columns, K=C=128),
  rhs = the matching 14x22 halo patch of y (N=308 columns).  Entry
  (m=(a,b), n=(al,be)) of the PSUM block is the correlation of x pixel
  (a,b) with y pixel (al-3, be-3) relative to the patch origin, so the 49
  shift planes live on 49 diagonals of each block.
* Diagonal extraction is not expressible with uniform per-partition access
  patterns on any engine, so each PSUM block is cast to fp16 and dumped
  whole to DRAM; the final banded gather is a cheap numpy fancy-index on
  the host.  Inputs are also shipped as fp16 (quantization error ~1e-3
  relative, well within tolerance), which keeps total HBM traffic per core
  at ~28 MB, close to the memory roofline.
"""

import numpy as np

import concourse.bass as bass  # noqa: F401  (AP types pulled in transitively)
import concourse.tile as tile
from concourse import bacc, mybir
from concourse.bass_utils import run_bass_kernel_spmd

B, C, H, W = 1, 128, 384, 512
NCORES = 8
HB = H // NCORES          # 48 output rows per core
PA, PB = 8, 16            # x patch: 8 rows x 16 cols = 128 = M
HA, HW_ = PA + 6, PB + 6  # y halo patch: 14 x 22
PR, PW = HB // PA, W // PB  # 6 patch-rows x 32 patch-cols = 192 patches
NF = HA * HW_             # 308 = N (matmul free size, <= 512)

F16 = mybir.dt.float16

_PROGRAM = None


def _build_program():
    nc = bacc.Bacc("TRN2", target_bir_lowering=False, debug=False)

    # x is pre-tiled on the host to [C, patch, m] so each patch's 128 weight
    # columns are contiguous (walrus requires a single free dim on the
    # stationary matmul operand).
    xb = nc.declare_dram_parameter("xb", [C, PR * PW, PA * PB], F16, isOutput=False)
    yb = nc.declare_dram_parameter("yb", [C, HB + 6, W + 6], F16, isOutput=False)
    corr = nc.declare_dram_parameter("corr", [PR, 128, PW, NF], F16, isOutput=True)

    with tile.TileContext(nc) as tc:
        with (
            tc.tile_pool(name="xpool", bufs=1) as xpool,
            tc.tile_pool(name="ypool", bufs=1) as ypool,
            tc.tile_pool(name="psum", bufs=4, space="PSUM") as psum_pool,
            tc.tile_pool(name="stage", bufs=2) as stage_pool,
        ):
            X = xpool.tile([C, PR * PW, PA * PB], F16)
            Y = ypool.tile([C, HB + 6, W + 6], F16)

            # Issue input loads in the order the patch-row pipeline consumes
            # them (the HW queue drains FIFO): patch-row pr needs X chunk pr
            # and Y rows [8pr, 8pr+14) = Y chunks pr and pr+1.
            def load_x(pr):
                nc.sync.dma_start(
                    X[:, pr * PW : (pr + 1) * PW, :], xb[:, pr * PW : (pr + 1) * PW, :]
                )

            def load_y(ch):  # Y chunk = 8 rows (last chunk 6 rows)
                r0, r1 = ch * 8, min(ch * 8 + 8, HB + 6)
                nc.sync.dma_start(Y[:, r0:r1, :], yb[:, r0:r1, :])

            load_x(0)
            load_y(0)
            load_y(1)
            for pr in range(1, PR):
                load_x(pr)
                load_y(pr + 1)

            for pr in range(PR):
                # One staging buffer and one output DMA per patch-row keeps
                # the Sync sequencer's per-DMA dispatch (~0.6us) off the
                # critical path.
                st = stage_pool.tile([128, PW, NF], F16)
                for wq in range(PW // 2):
                    # Two patches share one 2-bank PSUM tile; their
                    # evacuation is a single strided copy.
                    ps = psum_pool.tile([128, 2, 512], mybir.dt.float32)
                    for k in range(2):
                        wp = wq * 2 + k
                        lhsT = X[:, pr * PW + wp, :]
                        rhs = Y[:, pr * PA : pr * PA + HA, wp * PB : wp * PB + HW_]
                        nc.tensor.matmul(
                            ps[:, k, :NF], lhsT, rhs, start=True, stop=True
                        )
                    dst = st[:, wq * 2 : wq * 2 + 2, :]
                    # Alternate evacuation between DVE and ACT so neither
                    # becomes the bottleneck.
                    if wq % 2 == 0:
                        nc.vector.tensor_copy(dst, ps[:, :, :NF])
                    else:
                        nc.scalar.copy(dst, ps[:, :, :NF])
                    if wq == PW // 4 - 1:
                        # First half of the row band is done — ship it while
                        # the second half is still being computed.
                        nc.sync.dma_start(
                            corr[pr, :, : PW // 2], st[:, : PW // 2, :]
                        )
                nc.sync.dma_start(corr[pr, :, PW // 2 :], st[:, PW // 2 :, :])

    nc.compile()
    return nc


def _program():
    global _PROGRAM
    if _PROGRAM is None:
        _PROGRAM = _build_program()
    return _PROGRAM


def _make_in_maps(x: np.ndarray, y: np.ndarray):
    x0 = np.asarray(x[0]).astype(np.float16)
    # [C, H, W] -> [C, PRtot, PA, PW, PB] -> [C, PRtot, PW, PA, PB]
    xt = x0.reshape(C, H // PA, PA, PW, PB).transpose(0, 1, 3, 2, 4)
    xt = np.ascontiguousarray(xt.reshape(C, H // PA * PW, PA * PB))
    yp = np.zeros((C, H + 6, W + 6), np.float16)
    yp[:, 3 : 3 + H, 3 : 3 + W] = y[0]
    in_maps = []
    for k in range(NCORES):
        in_maps.append(
            {
                "xb": np.ascontiguousarray(xt[:, k * PR * PW : (k + 1) * PR * PW, :]),
                "yb": np.ascontiguousarray(yp[:, k * HB : k * HB + HB + 6, :]),
            }
        )
    return in_maps


_GATHER_IDX = None


def _gather_indices():
    global _GATHER_IDX
    if _GATHER_IDX is None:
        a = np.arange(PA)[:, None]
        b = np.arange(PB)[None, :]
        m_idx = a * PB + b  # [8, 16]
        j = np.arange(7)[:, None, None, None]
        i = np.arange(7)[None, :, None, None]
        n_idx = (a[None, None] + j) * HW_ + (b[None, None] + i)  # [7, 7, 8, 16]
        _GATHER_IDX = (m_idx[None, None], n_idx)
    return _GATHER_IDX


def _gather_core(corr_k: np.ndarray) -> np.ndarray:
    """[PR, 128, PW, NF] -> [49, HB, W] band of the output."""
    m_idx, n_idx = _gather_indices()
    # advanced indices (m, n) separated by a slice -> their axes come first
    g = corr_k[:, m_idx[0, 0], :, n_idx]  # [7, 7, 8, 16, PR, PW]
    g = g.transpose(0, 1, 4, 2, 5, 3).reshape(49, HB, W)
    return g


def _run(in_maps, trace=False, **kw):
    return run_bass_kernel_spmd(
        _program(), in_maps, core_ids=list(range(NCORES)), trace=trace, **kw
    )


def kernel(x: np.ndarray, y: np.ndarray) -> np.ndarray:
    x = np.asarray(x)
    y = np.asarray(y)
    res = _run(_make_in_maps(x, y)).results
    out = np.empty((1, 49, H, W), np.float32)
    for k in range(NCORES):
        out[0, :, k * HB : (k + 1) * HB, :] = _gather_core(
            np.asarray(res[k]["corr"])
        ).astype(np.float32)
    return out
